# revision 10
# baseline (speedup 1.0000x reference)
"""CenterLossLayer Trainium2 kernel — 8-core SPMD.

Math (reference):
    sel   = onehot @ centers                      # [B, D] — a row gather
    delta = onehot.T @ (sel - features)           # [C, D] — a scatter-add
    counts = onehot.sum(0) + 1                    # [C, 1]
    new_centers = centers - ALPHA * delta / counts
    loss = sum((features - sel)^2, axis=1)        # [B, 1]

Strategy (one-hot structure exploited, single NEFF, one AllGather):
  Phase 1 (batch-sharded, 512 rows/core): scan the f32 onehot rows with a
  fused multiply-by-iota max-reduce on DVE -> integer labels; indirect-DMA
  gather centers[labels] -> sel; diff + loss in fp32. Pack
  [diff_bf16 | label_hi | label_lo] rows into a [512, 258] bf16 buffer.
  Exchange: AllGather -> [4096, 258] bf16 on every core.
  Phase 3 (class-sharded, 1250 classes/core): rebuild one-hot tiles as bf16
  via is_equal(labels, class_iota); delta_ext = onehot.T @ [diff | 1] on PE
  (PSUM-accumulated over the 32 batch blocks; the ones column yields
  per-class counts); center update on the core's class shard.
Each core outputs its 512 loss rows and its 1250 rows of new_centers; the
host concatenates.
"""
import sys

import numpy as np

sys.path.insert(0, "/opt/trn_rl_repo")

import concourse.bass as bass  # noqa: E402
import concourse.tile as tile  # noqa: E402
from concourse import bacc, mybir  # noqa: E402
from concourse.bass import IndirectOffsetOnAxis  # noqa: E402
from concourse.bass_utils import run_bass_kernel_spmd  # noqa: E402

ALPHA = 0.5
B, C, D = 4096, 10000, 256
N_CORES = 8
BL = B // N_CORES          # 512 batch rows per core
CL = C // N_CORES          # 1250 classes per core
P = 128
NBLK = BL // P             # 4 local batch blocks
NGBLK = B // P             # 32 global batch blocks
NCHUNK = 4
CHUNK = C // NCHUNK        # 2500 scan chunk
EX = D + 2                 # exchange row: diff | label_hi | label_lo
F32 = mybir.dt.float32
BF16 = mybir.dt.bfloat16
I32 = mybir.dt.int32
AX = mybir.AxisListType
OP = mybir.AluOpType

_CACHE = {}

# Bisect knob: "p1" = phase 1 only, "ag" = phase 1 + AllGather readback,
# "full" = everything. Used while isolating HW-side failures.
import os  # noqa: E402
STAGE = os.environ.get("KERNEL_STAGE", "full")


def _build():
    nc = bacc.Bacc("TRN2", target_bir_lowering=False, debug=False,
                   num_devices=N_CORES)
    features_l = nc.dram_tensor("features_l", [BL, D], F32,
                                kind="ExternalInput").ap()
    onehot_l = nc.dram_tensor("onehot_l", [BL, C], F32,
                              kind="ExternalInput").ap()
    centers_full = nc.dram_tensor("centers_full", [C, D], F32,
                                  kind="ExternalInput").ap()
    centers_l = nc.dram_tensor("centers_l", [CL, D], F32,
                               kind="ExternalInput").ap()
    ciota_rep = nc.dram_tensor("ciota_rep", [P, CL], F32,
                               kind="ExternalInput").ap()
    loss_l = nc.dram_tensor("loss_l", [BL, 1], F32,
                            kind="ExternalOutput").ap()
    newc_l = nc.dram_tensor("newc_l", [CL, D], F32,
                            kind="ExternalOutput").ap()

    with tile.TileContext(nc) as tc:
        with tc.tile_pool(name="const", bufs=1) as constp, \
             tc.tile_pool(name="oh", bufs=3) as ohp, \
             tc.tile_pool(name="scr", bufs=2) as scrp, \
             tc.tile_pool(name="p1", bufs=2) as p1, \
             tc.tile_pool(name="p3", bufs=1) as p3, \
             tc.tile_pool(name="p3s", bufs=2) as p3s, \
             tc.tile_pool(name="recon", bufs=3) as reconp, \
             tc.tile_pool(name="psum", bufs=8, space="PSUM") as psp, \
             tc.tile_pool(name="dram", bufs=1, space="DRAM") as dram:

            # ---- static constants ----
            ones8 = constp.tile([P, 8], F32, name="ones8")
            nc.vector.memset(ones8[:], 1.0)
            bases = constp.tile([P, NCHUNK], F32, name="bases")
            nc.gpsimd.iota(bases[:], pattern=[[CHUNK, NCHUNK]], base=0,
                           channel_multiplier=0,
                           allow_small_or_imprecise_dtypes=True)
            ciota_sb = constp.tile([P, CL], F32, name="ciota_sb")
            nc.sync.dma_start(ciota_sb[:], ciota_rep[:])

            ag_in = dram.tile([BL, EX], BF16, name="ag_in")
            ag_out = dram.tile([B, EX], BF16, addr_space="Shared",
                               name="ag_out")

            # ---- phase 1: labels / sel / diff / loss on the batch shard ----
            for k in range(NBLK):
                rows = slice(k * P, (k + 1) * P)
                idx4 = p1.tile([P, NCHUNK * 8], mybir.dt.uint32, tag="idx4")
                for c in range(NCHUNK):
                    oh = ohp.tile([P, CHUNK], F32, tag="oh")
                    nc.sync.dma_start(
                        oh[:], onehot_l[rows, c * CHUNK:(c + 1) * CHUNK])
                    nc.vector.max_index(idx4[:, 8 * c:8 * c + 8], ones8[:],
                                        oh[:])
                idxf = p1.tile([P, NCHUNK], F32, tag="idxf")
                nc.vector.tensor_copy(idxf[:], idx4[:, 0:NCHUNK * 8:8])
                cand = p1.tile([P, NCHUNK], F32, tag="cand")
                nc.vector.tensor_add(cand[:], idxf[:], bases[:])
                labels_f = p1.tile([P, 1], F32, tag="labels_f")
                nc.vector.tensor_reduce(labels_f[:], cand[:], axis=AX.X,
                                        op=OP.min)
                labels_i = p1.tile([P, 1], I32, tag="labels_i")
                nc.vector.tensor_copy(labels_i[:], labels_f[:])

                sel = p1.tile([P, D], F32, tag="sel")
                nc.gpsimd.indirect_dma_start(
                    out=sel[:], out_offset=None, in_=centers_full[:],
                    in_offset=IndirectOffsetOnAxis(ap=labels_i[:, :1], axis=0))
                feat = p1.tile([P, D], F32, tag="feat")
                nc.sync.dma_start(feat[:], features_l[rows, :])
                diff = p1.tile([P, D], F32, tag="diff")
                nc.vector.tensor_sub(diff[:], sel[:], feat[:])

                sq = p1.tile([P, D], F32, tag="sq")
                loss_t = p1.tile([P, 1], F32, tag="loss_t")
                nc.scalar.activation(out=sq[:], in_=diff[:],
                                     func=mybir.ActivationFunctionType.Square)
                nc.vector.reduce_sum(loss_t[:], sq[:], axis=AX.X)
                nc.sync.dma_start(loss_l[rows, :], loss_t[:])

                exch = p1.tile([P, EX], BF16, tag="exch")
                nc.vector.tensor_copy(exch[:, 0:D], diff[:])
                hi_i = p1.tile([P, 1], I32, tag="hi_i")
                nc.vector.tensor_scalar(out=hi_i[:], in0=labels_i[:],
                                        scalar1=7, scalar2=None,
                                        op0=OP.arith_shift_right)
                lo_i = p1.tile([P, 1], I32, tag="lo_i")
                nc.vector.tensor_scalar(out=lo_i[:], in0=labels_i[:],
                                        scalar1=127, scalar2=None,
                                        op0=OP.bitwise_and)
                nc.vector.tensor_copy(exch[:, D:D + 1], hi_i[:])
                nc.vector.tensor_copy(exch[:, D + 1:D + 2], lo_i[:])
                nc.sync.dma_start(ag_in[rows, :], exch[:])

            # ---- exchange ----
            if STAGE != "p1":
                nc.gpsimd.collective_compute(
                    "AllGather", OP.bypass,
                    replica_groups=[list(range(N_CORES))],
                    ins=[ag_in.opt()], outs=[ag_out.opt()])

            # ---- phase 3 staging: full diff/labels into SBUF ----
            rhs_all = p3.tile([P, NGBLK * EX], BF16, name="rhs_all")
            labs_all = p3.tile([P, NGBLK], F32, name="labs_all")
            for g in range(NGBLK if STAGE == "full" else 0):
                b0 = g * EX
                nc.sync.dma_start(rhs_all[:, b0:b0 + EX],
                                  ag_out[g * P:(g + 1) * P, :])
                t_hi = p3s.tile([P, 1], F32, tag="t_hi")
                nc.vector.tensor_scalar(out=t_hi[:],
                                        in0=rhs_all[:, b0 + D:b0 + D + 1],
                                        scalar1=128.0, scalar2=None,
                                        op0=OP.mult)
                nc.vector.tensor_tensor(out=labs_all[:, g:g + 1], in0=t_hi[:],
                                        in1=rhs_all[:, b0 + D + 1:b0 + D + 2],
                                        op=OP.add)
                nc.vector.memset(rhs_all[:, b0 + D:b0 + D + 1], 1.0)

            # ---- phase 3: delta matmuls + update, two PSUM half-shards ----
            for c0, c1 in (((0, 640), (640, CL)) if STAGE == "full" else ()):
                width = c1 - c0
                mts = [(m0, min(P, width - m0)) for m0 in range(0, width, P)]
                psums = [psp.tile([P, D + 1], F32, tag="delta_ps",
                                  name=f"ps_{c0}_{i}")
                         for i in range(len(mts))]
                for g in range(NGBLK):
                    recon = reconp.tile([P, 640], BF16, tag="recon")
                    nc.vector.tensor_tensor(
                        out=recon[:, :width],
                        in0=labs_all[:, g:g + 1].to_broadcast([P, width]),
                        in1=ciota_sb[:, c0:c1], op=OP.is_equal)
                    for i, (m0, msz) in enumerate(mts):
                        nc.tensor.matmul(
                            out=psums[i][:msz, :],
                            lhsT=recon[:, m0:m0 + msz],
                            rhs=rhs_all[:, g * EX:g * EX + D + 1],
                            start=(g == 0), stop=(g == NGBLK - 1))
                for i, (m0, msz) in enumerate(mts):
                    mabs = c0 + m0
                    cnt1 = p3s.tile([P, 1], F32, tag="cnt1")
                    nc.vector.tensor_scalar_add(
                        cnt1[:msz], psums[i][:msz, D:D + 1], 1.0)
                    recip = p3s.tile([P, 1], F32, tag="recip")
                    nc.vector.reciprocal(recip[:msz], cnt1[:msz])
                    recs = p3s.tile([P, 1], F32, tag="recs")
                    nc.vector.tensor_scalar_mul(recs[:msz], recip[:msz],
                                                -ALPHA)
                    cen = p3s.tile([P, D], F32, tag="cen")
                    nc.sync.dma_start(cen[:msz], centers_l[mabs:mabs + msz, :])
                    upd = p3s.tile([P, D], F32, tag="upd")
                    nc.vector.tensor_scalar(out=upd[:msz],
                                            in0=psums[i][:msz, 0:D],
                                            scalar1=recs[:msz, :1],
                                            scalar2=None, op0=OP.mult)
                    newc = p3s.tile([P, D], F32, tag="newc")
                    nc.vector.tensor_add(newc[:msz], cen[:msz], upd[:msz])
                    nc.sync.dma_start(newc_l[mabs:mabs + msz, :], newc[:msz])
    nc.compile()
    return nc


def _get_nc():
    if "nc" not in _CACHE:
        _CACHE["nc"] = _build()
    return _CACHE["nc"]


def kernel(features, onehot, centers):
    features = np.ascontiguousarray(features, dtype=np.float32)
    onehot = np.ascontiguousarray(onehot, dtype=np.float32)
    centers = np.ascontiguousarray(centers, dtype=np.float32)
    nc = _get_nc()

    in_maps = []
    for i in range(N_CORES):
        ciota = np.broadcast_to(
            np.arange(i * CL, (i + 1) * CL, dtype=np.float32)[None, :],
            (P, CL)).copy()
        in_maps.append({
            "features_l": features[i * BL:(i + 1) * BL],
            "onehot_l": onehot[i * BL:(i + 1) * BL],
            "centers_full": centers,
            "centers_l": centers[i * CL:(i + 1) * CL],
            "ciota_rep": ciota,
        })
    res = run_bass_kernel_spmd(nc, in_maps, core_ids=list(range(N_CORES)))
    loss = np.concatenate([res.results[i]["loss_l"] for i in range(N_CORES)],
                          axis=0)
    new_centers = np.concatenate(
        [res.results[i]["newc_l"] for i in range(N_CORES)], axis=0)
    return loss, new_centers


# revision 11
# speedup vs baseline: 1.0012x; 1.0012x over previous
"""CenterLossLayer Trainium2 kernel — 8-core SPMD.

Math (reference):
    sel   = onehot @ centers                      # [B, D] — a row gather
    delta = onehot.T @ (sel - features)           # [C, D] — a scatter-add
    counts = onehot.sum(0) + 1                    # [C, 1]
    new_centers = centers - ALPHA * delta / counts
    loss = sum((features - sel)^2, axis=1)        # [B, 1]

Strategy (one-hot structure exploited, pipelined collectives):
  Phase 1 (batch-sharded, 512 rows/core, 4 blocks of 128): find each row's
  label with DVE max_index over the f32 onehot (the row max is known to be
  1.0), chunked so DMA overlaps the scan; indirect-DMA gather
  centers[labels] -> sel; diff + loss in fp32. Pack
  [diff_bf16 | label_hi | label_lo] into a [128, 258] bf16 block and
  AllGather it immediately (4 pipelined collectives, one per block).
  Phase 3 (class-sharded, 1250 classes/core): as each gathered chunk lands,
  rebuild its one-hot tile in bf16 — on ScalarE via Relu(1-(c-l)^2) after a
  Square pass (exact for integer labels), or on VectorE via is_equal, split
  to balance engines — and accumulate delta_ext = onehot.T @ [diff | 1] on
  PE into PSUM (the ones column yields per-class counts). Classes are
  processed in two 625-wide halves (5 PSUM banks each): half A is pipelined
  with the collectives, half B reruns over the SBUF-resident chunks.
Each core outputs its 512 loss rows and its 1250 rows of new_centers; the
host concatenates.
"""
import os
import sys

import numpy as np

sys.path.insert(0, "/opt/trn_rl_repo")

import concourse.bass as bass  # noqa: E402
import concourse.tile as tile  # noqa: E402
from concourse import bacc, mybir  # noqa: E402
from concourse.bass import IndirectOffsetOnAxis  # noqa: E402
from concourse.bass_utils import run_bass_kernel_spmd  # noqa: E402

ALPHA = 0.5
B, C, D = 4096, 10000, 256
N_CORES = 8
BL = B // N_CORES          # 512 batch rows per core
CL = C // N_CORES          # 1250 classes per core
P = 128
NBLK = BL // P             # 4 local batch blocks
NGBLK = B // P             # 32 global 128-row chunks
NCHUNK = 4
CHUNK = C // NCHUNK        # 2500 scan chunk
EX = D + 2                 # exchange row: diff | label_hi | label_lo
HALF = 625                 # class half-shard (5 m-tiles of 125)
F32 = mybir.dt.float32
BF16 = mybir.dt.bfloat16
I32 = mybir.dt.int32
U32 = mybir.dt.uint32
AX = mybir.AxisListType
OP = mybir.AluOpType
AF = mybir.ActivationFunctionType

_CACHE = {}
STAGE = os.environ.get("KERNEL_STAGE", "full")
# fraction of one-hot reconstruction units routed to ScalarE (2 passes each)
ACT_RECON_MOD = 4          # chunk % 4 == 3 -> VectorE, else ScalarE


def _recon(nc, pools, labs_all, neg_labs, ciota_sb, g, c0, use_act):
    """Rebuild bf16 one-hot tile [P, HALF] for gathered chunk g, classes
    [c0, c0+HALF) of this core's shard."""
    reconp, sqp = pools
    recon = reconp.tile([P, HALF], BF16, tag="recon", name=f"recon_{c0}_{g}")
    if use_act:
        sq = sqp.tile([P, HALF], F32, tag="sq_scr", name=f"sq_{c0}_{g}")
        nc.scalar.activation(out=sq[:], in_=ciota_sb[:, c0:c0 + HALF],
                             func=AF.Square, bias=neg_labs[:, g:g + 1],
                             scale=1.0)
        nc.scalar.activation(out=recon[:], in_=sq[:], func=AF.Relu,
                             bias=1.0, scale=-1.0)
    else:
        nc.vector.tensor_tensor(
            out=recon[:],
            in0=labs_all[:, g:g + 1].to_broadcast([P, HALF]),
            in1=ciota_sb[:, c0:c0 + HALF], op=OP.is_equal)
    return recon


def _build():
    nc = bacc.Bacc("TRN2", target_bir_lowering=False, debug=False,
                   num_devices=N_CORES)
    features_l = nc.dram_tensor("features_l", [BL, D], F32,
                                kind="ExternalInput").ap()
    onehot_l = nc.dram_tensor("onehot_l", [BL, C], F32,
                              kind="ExternalInput").ap()
    centers_full = nc.dram_tensor("centers_full", [C, D], F32,
                                  kind="ExternalInput").ap()
    centers_l = nc.dram_tensor("centers_l", [CL, D], F32,
                               kind="ExternalInput").ap()
    ciota_rep = nc.dram_tensor("ciota_rep", [P, CL], F32,
                               kind="ExternalInput").ap()
    loss_l = nc.dram_tensor("loss_l", [BL, 1], F32,
                            kind="ExternalOutput").ap()
    newc_l = nc.dram_tensor("newc_l", [CL, D], F32,
                            kind="ExternalOutput").ap()

    with tile.TileContext(nc) as tc:
        with tc.tile_pool(name="const", bufs=1) as constp, \
             tc.tile_pool(name="oh", bufs=5) as ohp, \
             tc.tile_pool(name="p1", bufs=2) as p1, \
             tc.tile_pool(name="p3", bufs=1) as p3, \
             tc.tile_pool(name="p3s", bufs=2) as p3s, \
             tc.tile_pool(name="recon", bufs=4) as reconp, \
             tc.tile_pool(name="sqs", bufs=4) as sqp, \
             tc.tile_pool(name="psum", bufs=8, space="PSUM") as psp, \
             tc.tile_pool(name="dram", bufs=1, space="DRAM") as dram:

            # ---- static constants ----
            ones8 = constp.tile([P, 8], F32, name="ones8")
            nc.vector.memset(ones8[:], 1.0)
            bases = constp.tile([P, NCHUNK], F32, name="bases")
            nc.gpsimd.iota(bases[:], pattern=[[CHUNK, NCHUNK]], base=0,
                           channel_multiplier=0,
                           allow_small_or_imprecise_dtypes=True)
            ciota_sb = constp.tile([P, CL], F32, name="ciota_sb")
            nc.sync.dma_start(ciota_sb[:], ciota_rep[:])

            ag_ins = [dram.tile([P, EX], BF16, name=f"ag_in{k}")
                      for k in range(NBLK)]
            ag_outs = [dram.tile([N_CORES * P, EX], BF16, addr_space="Shared",
                                 name=f"ag_out{k}") for k in range(NBLK)]

            # phase-3 SBUF-resident gathered data
            rhs_all = p3.tile([P, NGBLK * EX], BF16, name="rhs_all")
            labs_all = p3.tile([P, NGBLK], F32, name="labs_all")
            neg_labs = p3.tile([P, NGBLK], F32, name="neg_labs")

            # ---- phase 1 + per-block AllGather ----
            for k in range(NBLK):
                rows = slice(k * P, (k + 1) * P)
                idx4 = p1.tile([P, NCHUNK * 8], U32, tag="idx4")
                for c in range(NCHUNK):
                    oh = ohp.tile([P, CHUNK], F32, tag="oh")
                    nc.sync.dma_start(
                        oh[:], onehot_l[rows, c * CHUNK:(c + 1) * CHUNK])
                    nc.vector.max_index(idx4[:, 8 * c:8 * c + 8], ones8[:],
                                        oh[:])
                idxf = p1.tile([P, NCHUNK], F32, tag="idxf")
                nc.vector.tensor_copy(idxf[:], idx4[:, 0:NCHUNK * 8:8])
                cand = p1.tile([P, NCHUNK], F32, tag="cand")
                nc.vector.tensor_add(cand[:], idxf[:], bases[:])
                labels_f = p1.tile([P, 1], F32, tag="labels_f")
                nc.vector.tensor_reduce(labels_f[:], cand[:], axis=AX.X,
                                        op=OP.min)
                labels_i = p1.tile([P, 1], I32, tag="labels_i")
                nc.vector.tensor_copy(labels_i[:], labels_f[:])

                sel = p1.tile([P, D], F32, tag="sel")
                nc.gpsimd.indirect_dma_start(
                    out=sel[:], out_offset=None, in_=centers_full[:],
                    in_offset=IndirectOffsetOnAxis(ap=labels_i[:, :1], axis=0))
                feat = p1.tile([P, D], F32, tag="feat")
                nc.sync.dma_start(feat[:], features_l[rows, :])
                diff = p1.tile([P, D], F32, tag="diff")
                nc.vector.tensor_sub(diff[:], sel[:], feat[:])

                sq = p1.tile([P, D], F32, tag="sq")
                loss_t = p1.tile([P, 1], F32, tag="loss_t")
                nc.scalar.activation(out=sq[:], in_=diff[:], func=AF.Square)
                nc.vector.reduce_sum(loss_t[:], sq[:], axis=AX.X)
                nc.sync.dma_start(loss_l[rows, :], loss_t[:])

                exch = p1.tile([P, EX], BF16, tag="exch")
                nc.vector.tensor_copy(exch[:, 0:D], diff[:])
                hi_i = p1.tile([P, 1], I32, tag="hi_i")
                nc.vector.tensor_scalar(out=hi_i[:], in0=labels_i[:],
                                        scalar1=7, scalar2=None,
                                        op0=OP.arith_shift_right)
                lo_i = p1.tile([P, 1], I32, tag="lo_i")
                nc.vector.tensor_scalar(out=lo_i[:], in0=labels_i[:],
                                        scalar1=127, scalar2=None,
                                        op0=OP.bitwise_and)
                nc.vector.tensor_copy(exch[:, D:D + 1], hi_i[:])
                nc.vector.tensor_copy(exch[:, D + 1:D + 2], lo_i[:])
                nc.sync.dma_start(ag_ins[k][:], exch[:])

                if STAGE != "p1":
                    nc.gpsimd.collective_compute(
                        "AllGather", OP.bypass,
                        replica_groups=[list(range(N_CORES))],
                        ins=[ag_ins[k].opt()], outs=[ag_outs[k].opt()])

            if STAGE == "p1":
                nc.compile()
                return nc

            # ---- phase 3: stage chunks, half-A recon+matmul (pipelined) ----
            mts = [(m0, min(P, HALF - m0)) for m0 in range(0, HALF, P)]
            psA = [psp.tile([P, D + 1], F32, tag="delta_ps", name=f"psA_{i}")
                   for i in range(len(mts))]
            for g in range(NGBLK):
                k, j = divmod(g, N_CORES)
                b0 = g * EX
                nc.sync.dma_start(rhs_all[:, b0:b0 + EX],
                                  ag_outs[k][j * P:(j + 1) * P, :])
                t_hi = p3s.tile([P, 1], F32, tag="t_hi")
                nc.vector.tensor_scalar(out=t_hi[:],
                                        in0=rhs_all[:, b0 + D:b0 + D + 1],
                                        scalar1=128.0, scalar2=None,
                                        op0=OP.mult)
                nc.vector.tensor_tensor(out=labs_all[:, g:g + 1], in0=t_hi[:],
                                        in1=rhs_all[:, b0 + D + 1:b0 + D + 2],
                                        op=OP.add)
                nc.vector.tensor_scalar(out=neg_labs[:, g:g + 1],
                                        in0=labs_all[:, g:g + 1],
                                        scalar1=-1.0, scalar2=None,
                                        op0=OP.mult)
                nc.vector.memset(rhs_all[:, b0 + D:b0 + D + 1], 1.0)

                recon = _recon(nc, (reconp, sqp), labs_all, neg_labs,
                               ciota_sb, g, 0, g % ACT_RECON_MOD != 3)
                for i, (m0, msz) in enumerate(mts):
                    nc.tensor.matmul(
                        out=psA[i][:msz, :], lhsT=recon[:, m0:m0 + msz],
                        rhs=rhs_all[:, b0:b0 + D + 1],
                        start=(g == 0), stop=(g == NGBLK - 1))

            # ---- half-B: second pass over SBUF-resident chunks ----
            psB = [psp.tile([P, D + 1], F32, tag="delta_ps", name=f"psB_{i}")
                   for i in range(len(mts))]
            for g in range(NGBLK):
                b0 = g * EX
                recon = _recon(nc, (reconp, sqp), labs_all, neg_labs,
                               ciota_sb, g, HALF, g % ACT_RECON_MOD != 3)
                for i, (m0, msz) in enumerate(mts):
                    nc.tensor.matmul(
                        out=psB[i][:msz, :], lhsT=recon[:, m0:m0 + msz],
                        rhs=rhs_all[:, b0:b0 + D + 1],
                        start=(g == 0), stop=(g == NGBLK - 1))

            # ---- update: new_centers = centers - ALPHA*delta/(counts+1) ----
            for c0, psums in ((0, psA), (HALF, psB)):
                for i, (m0, msz) in enumerate(mts):
                    mabs = c0 + m0
                    cnt1 = p3s.tile([P, 1], F32, tag="cnt1")
                    nc.vector.tensor_scalar_add(
                        cnt1[:msz], psums[i][:msz, D:D + 1], 1.0)
                    recip = p3s.tile([P, 1], F32, tag="recip")
                    nc.vector.reciprocal(recip[:msz], cnt1[:msz])
                    recs = p3s.tile([P, 1], F32, tag="recs")
                    nc.vector.tensor_scalar_mul(recs[:msz], recip[:msz],
                                                -ALPHA)
                    cen = p3s.tile([P, D], F32, tag="cen")
                    nc.sync.dma_start(cen[:msz], centers_l[mabs:mabs + msz, :])
                    upd = p3s.tile([P, D], F32, tag="upd")
                    nc.vector.tensor_scalar(out=upd[:msz],
                                            in0=psums[i][:msz, 0:D],
                                            scalar1=recs[:msz, :1],
                                            scalar2=None, op0=OP.mult)
                    newc = p3s.tile([P, D], F32, tag="newc")
                    nc.vector.tensor_add(newc[:msz], cen[:msz], upd[:msz])
                    nc.sync.dma_start(newc_l[mabs:mabs + msz, :], newc[:msz])
    nc.compile()
    return nc


def _get_nc():
    if "nc" not in _CACHE:
        _CACHE["nc"] = _build()
    return _CACHE["nc"]


def _in_maps(features, onehot, centers):
    maps = []
    for i in range(N_CORES):
        ciota = np.broadcast_to(
            np.arange(i * CL, (i + 1) * CL, dtype=np.float32)[None, :],
            (P, CL)).copy()
        maps.append({
            "features_l": features[i * BL:(i + 1) * BL],
            "onehot_l": onehot[i * BL:(i + 1) * BL],
            "centers_full": centers,
            "centers_l": centers[i * CL:(i + 1) * CL],
            "ciota_rep": ciota,
        })
    return maps


def kernel(features, onehot, centers):
    features = np.ascontiguousarray(features, dtype=np.float32)
    onehot = np.ascontiguousarray(onehot, dtype=np.float32)
    centers = np.ascontiguousarray(centers, dtype=np.float32)
    nc = _get_nc()
    res = run_bass_kernel_spmd(nc, _in_maps(features, onehot, centers),
                               core_ids=list(range(N_CORES)))
    loss = np.concatenate([res.results[i]["loss_l"] for i in range(N_CORES)],
                          axis=0)
    new_centers = np.concatenate(
        [res.results[i]["newc_l"] for i in range(N_CORES)], axis=0)
    return loss, new_centers
